# revision 7
# baseline (speedup 1.0000x reference)
"""Additive (Bahdanau) attention on 8 TRN2 NeuronCores — self-contained Bass kernel.

Math: score(q,k) = w2 . tanh(hq[q] + hk[k] + b1) + b2;  out = softmax_k(score) @ V.

Key restructuring: tanh(s) is approximated by an M-term sine series
    tanh(s) ~= sum_m c_m sin(w_m s),   w_m = m*pi/L  (L=10)
(max-err ~2e-2/7.6e-3 for M=10/12 on [-7,7]; |s| <= ~6.5 here).  Then
    sin(w(a+b)) = sin(wa)cos(wb) + cos(wa)sin(wb)
turns the whole [B,Q,K,D] tanh+reduce into a TensorE matmul with contraction
over (2M x D):  logits^T[k,q] = sum_{m,d} G[(m,d),k] * F[(m,d),q].

HW Sin is only accurate on [-pi, pi], so arguments are range-reduced on the
VectorE with a mod: weights are pre-scaled by 1/(2L) so hq/hk arrive in
"turns"; per frequency m: t = m*u + phase (+8 keeps t positive), f = mod(t,1),
then ScalarE computes sin(2*pi*f - pi) = -sin(2*pi*t).  Both factors of every
product carry the -1, so the signs cancel.  b2 drops (softmax shift
invariance); b1 and the cos phase fold into the affine; w2 and c_m fold into a
per-partition scale of F; 1/denominator folds into the final per-q scaling of
attn@V (denominator via a ones-matmul, reciprocal on VectorE).

Sharding: data-parallel over batch, B=16 -> 2 per core, no collectives.
"""

import math
from contextlib import ExitStack

import numpy as np

import concourse.bass as bass
import concourse.mybir as mybir
import concourse.tile as tile
from concourse import bacc
from concourse.bass_utils import run_bass_kernel_spmd
from concourse.masks import make_identity

F32 = mybir.dt.float32
BF16 = mybir.dt.bfloat16
AF = mybir.ActivationFunctionType
ALU = mybir.AluOpType

NCORES = 8
B, NQ, NK, D = 16, 256, 256, 256
BL = B // NCORES          # local batches per core = 2
P = 128
DC = D // P               # d-chunks = 2
EC = D // P               # e-chunks (contraction for hq/hk matmuls) = 2
QT = NQ // P              # q-tiles = 2
KT = NK // P              # k-tiles = 2
M_SINES = 6
TWO_PI = 2.0 * math.pi
MAGIC = 12582912.0        # 1.5 * 2**23: fp32 add/sub rounds to nearest integer

# Free-frequency weighted-LSQ fit of tanh on [-8.5, 8.5] (Gaussian(1.05)+1e-3
# weight); max err 1.1e-2 on [-7,7], rms 3.4e-4 under the N(0,1) input law.
OMEGA = np.array([0.3126234509, 0.9450487939, 1.5943044008,
                  2.2642467516, 2.9676298346, 4.0923054586])
COEF = np.array([1.2258418724, 0.3056790312, 0.1089743779,
                 0.0378162937, 0.0157983516, 0.0046812861])
NU = OMEGA / TWO_PI       # "turns" multiplier


def build_kernel() -> bacc.Bacc:
    nc = bacc.Bacc("TRN2", target_bir_lowering=False, debug=False)

    q_d = nc.dram_tensor("queries", [BL, NQ, D], F32, kind="ExternalInput").ap()
    k_d = nc.dram_tensor("keys", [BL, NK, D], F32, kind="ExternalInput").ap()
    v_d = nc.dram_tensor("values", [BL, NK, D], F32, kind="ExternalInput").ap()
    wq_d = nc.dram_tensor("Wq", [D, D], F32, kind="ExternalInput").ap()
    wk_d = nc.dram_tensor("Wk", [D, D], F32, kind="ExternalInput").ap()
    phaseg_d = nc.dram_tensor("phaseg", [P, M_SINES * DC], F32, kind="ExternalInput").ap()
    w2c_d = nc.dram_tensor("w2c", [P, M_SINES * DC], F32, kind="ExternalInput").ap()
    out_d = nc.dram_tensor("out", [BL, NQ, D], F32, kind="ExternalOutput").ap()

    with tile.TileContext(nc) as tc, ExitStack() as ctx:
        cpool = ctx.enter_context(tc.tile_pool(name="consts", bufs=1))
        dpool = ctx.enter_context(tc.tile_pool(name="data", bufs=1))

        ident = cpool.tile([P, P], F32)
        make_identity(nc, ident[:])
        ones_bf = cpool.tile([P, 1], BF16)
        nc.gpsimd.memset(ones_bf[:], 1.0)
        halfpi = cpool.tile([P, 1], F32)
        nc.gpsimd.memset(halfpi[:], math.pi / 2.0)

        wq_sb = cpool.tile([P, EC * D], F32)
        wk_sb = cpool.tile([P, EC * D], F32)
        for ec in range(EC):
            nc.sync.dma_start(wq_sb[:, ec * D:(ec + 1) * D], wq_d[ec * P:(ec + 1) * P, :])
            nc.sync.dma_start(wk_sb[:, ec * D:(ec + 1) * D], wk_d[ec * P:(ec + 1) * P, :])
        phaseg = cpool.tile([P, M_SINES * DC], F32)
        nc.sync.dma_start(phaseg[:], phaseg_d[:])
        w2c = cpool.tile([P, M_SINES * DC], F32)
        nc.sync.dma_start(w2c[:], w2c_d[:])

        # natural-layout loads: col = (b*2 + tile)*256 + inner
        qn = dpool.tile([P, BL * QT * D], F32)
        kn = dpool.tile([P, BL * KT * D], F32)
        vn = dpool.tile([P, BL * KT * D], F32)
        for b in range(BL):
            for t in range(QT):
                nc.sync.dma_start(qn[:, (b * QT + t) * D:(b * QT + t + 1) * D],
                                  q_d[b, t * P:(t + 1) * P, :])
            for t in range(KT):
                nc.sync.dma_start(kn[:, (b * KT + t) * D:(b * KT + t + 1) * D],
                                  k_d[b, t * P:(t + 1) * P, :])
                nc.sync.dma_start(vn[:, (b * KT + t) * D:(b * KT + t + 1) * D],
                                  v_d[b, t * P:(t + 1) * P, :])

        # transposed inputs: col = (ec*BL + b)*256 + q, partition = e-in-chunk
        qTt = dpool.tile([P, EC * BL * NQ], F32)
        kTt = dpool.tile([P, EC * BL * NK], F32)

        # u = h/(2L) in "turns": [d-in-dtile (part), b*256 + q (free)]
        hq_sb = [dpool.tile([P, BL * NQ], F32, name=f"hq_sb{i}", tag=f"hq_sb{i}") for i in range(DC)]
        hk_sb = [dpool.tile([P, BL * NK], F32, name=f"hk_sb{i}", tag=f"hk_sb{i}") for i in range(DC)]

        with tc.tile_pool(name="hpsum", bufs=4, space="PSUM") as hpool:
            with tc.tile_pool(name="tpsum", bufs=2, space="PSUM") as tpool:
                for (src, dst, nt) in ((qn, qTt, QT), (kn, kTt, KT)):
                    for b in range(BL):
                        for i in range(nt):
                            for j in range(EC):
                                tp = tpool.tile([P, P], F32)
                                nc.tensor.transpose(
                                    tp[:],
                                    src[:, (b * nt + i) * D + j * P:(b * nt + i) * D + (j + 1) * P],
                                    ident[:])
                                nc.vector.tensor_copy(
                                    dst[:, (j * BL + b) * NQ + i * P:(j * BL + b) * NQ + (i + 1) * P],
                                    tp[:])

            for (w_sb, srcT, h_sb, n) in ((wq_sb, qTt, hq_sb, NQ), (wk_sb, kTt, hk_sb, NK)):
                for dt in range(DC):
                    h_ps = hpool.tile([P, BL * n], F32)
                    for b in range(BL):
                        for ec in range(EC):
                            nc.tensor.matmul(
                                h_ps[:, b * n:(b + 1) * n],
                                w_sb[:, ec * D + dt * P:ec * D + (dt + 1) * P],
                                srcT[:, (ec * BL + b) * n:(ec * BL + b + 1) * n],
                                start=(ec == 0), stop=(ec == EC - 1))
                    nc.vector.tensor_copy(h_sb[dt][:], h_ps[:])

        wpool = ctx.enter_context(tc.tile_pool(name="wpsum", bufs=4, space="PSUM"))
        dnpool = ctx.enter_context(tc.tile_pool(name="dnpsum", bufs=2, space="PSUM"))
        tfpool = ctx.enter_context(tc.tile_pool(name="turns", bufs=4))
        frpool = ctx.enter_context(tc.tile_pool(name="fracs", bufs=6))
        rpool = ctx.enter_context(tc.tile_pool(name="raws", bufs=8))
        s1pool = ctx.enter_context(tc.tile_pool(name="scaledF", bufs=4))

        # logits^T accumulation: tile per (k-tile, batch) — a PSUM accumulation
        # group claims a whole 2KB bank, so concurrent groups get separate tiles
        logits_ps = [[wpool.tile([P, NQ], F32, name=f"lg_{kt}_{b}", tag="work")
                      for b in range(BL)] for kt in range(KT)]

        first = True
        for mi in range(M_SINES):
            for dt in range(DC):
                col = mi * DC + dt
                last = (mi == M_SINES - 1) and (dt == DC - 1)
                nu = float(NU[mi])
                sides = []
                for (tag, h_sb_t, ph) in (("F", hq_sb[dt], None), ("G", hk_sb[dt], phaseg)):
                    # t = nu*h (+ nu*b1 on the key side); fs = t - round(t)
                    t = tfpool.tile([P, BL * NQ], F32, name=f"t{tag}", tag=f"t{tag}")
                    if ph is None:
                        nc.vector.tensor_scalar(t[:], h_sb_t[:], nu, None, op0=ALU.mult)
                    else:
                        nc.vector.tensor_scalar(t[:], h_sb_t[:], nu, ph[:, col:col + 1],
                                                op0=ALU.mult, op1=ALU.add)
                    r = tfpool.tile([P, BL * NQ], F32, name=f"r{tag}", tag=f"r{tag}")
                    nc.vector.tensor_scalar(r[:], t[:], MAGIC, MAGIC,
                                            op0=ALU.add, op1=ALU.subtract)
                    fs = frpool.tile([P, BL * NQ], F32, name=f"fs{tag}", tag=f"fs{tag}")
                    nc.vector.tensor_tensor(fs[:], t[:], r[:], op=ALU.subtract)
                    # sin(2*pi*fs) directly; cos(2*pi*fs) = sin(pi/2 - |2*pi*fs|)
                    sn = rpool.tile([P, BL * NQ], BF16, name=f"sn{tag}", tag=f"sn{tag}")
                    nc.scalar.activation(sn[:], fs[:], AF.Sin, bias=0.0, scale=TWO_PI)
                    ab = frpool.tile([P, BL * NQ], F32, name=f"ab{tag}", tag=f"ab{tag}")
                    nc.scalar.activation(ab[:], fs[:], AF.Abs, bias=0.0, scale=TWO_PI)
                    cs = rpool.tile([P, BL * NQ], BF16, name=f"cs{tag}", tag=f"cs{tag}")
                    nc.scalar.activation(cs[:], ab[:], AF.Sin, bias=halfpi[:], scale=-1.0)
                    sides.append((sn, cs))
                (rFs, rFc), (rGs, rGc) = sides
                sFs = s1pool.tile([P, BL * NQ], BF16)
                nc.vector.tensor_scalar_mul(sFs[:], rFs[:], w2c[:, col:col + 1])
                sFc = s1pool.tile([P, BL * NQ], BF16)
                nc.vector.tensor_scalar_mul(sFc[:], rFc[:], w2c[:, col:col + 1])
                # logits += Gcos^T Fsin + Gsin^T Fcos   (per batch, per k-tile)
                for (gt, ft) in ((rGc, sFs), (rGs, sFc)):
                    for b in range(BL):
                        for kt in range(KT):
                            nc.tensor.matmul(
                                logits_ps[kt][b][:],
                                gt[:, b * NK + kt * P:b * NK + (kt + 1) * P],
                                ft[:, b * NQ:(b + 1) * NQ],
                                start=first, stop=(last and gt is rGs))
                    first = False

        # exp(logits^T) -> bf16 SBUF, col = (kt*BL + b)*256 + q
        expT = dpool.tile([P, KT * BL * NQ], BF16)
        for kt in range(KT):
            for b in range(BL):
                nc.scalar.activation(
                    expT[:, (kt * BL + b) * NQ:(kt * BL + b + 1) * NQ],
                    logits_ps[kt][b][:], AF.Exp)

        # denominators as columns via ones-matmul (one bank per accumulation group)
        recip_sb = cpool.tile([P, BL * QT], F32)
        for b in range(BL):
            for qt in range(QT):
                dn = dnpool.tile([P, 1], F32, name=f"dn_{b}_{qt}", tag="dn")
                for kt in range(KT):
                    nc.tensor.matmul(
                        dn[:],
                        expT[:, (kt * BL + b) * NQ + qt * P:(kt * BL + b) * NQ + (qt + 1) * P],
                        ones_bf[:],
                        start=(kt == 0), stop=(kt == KT - 1))
                nc.vector.reciprocal(recip_sb[:, b * QT + qt:b * QT + qt + 1], dn[:])

        vb = dpool.tile([P, BL * KT * D], BF16)
        nc.vector.tensor_copy(vb[:], vn[:])

        # attn @ V (unnormalized), then fold in 1/denom per q-partition
        out_sb = dpool.tile([P, BL * QT * D], F32)
        for qt in range(QT):
            for b in range(BL):
                av_ps = wpool.tile([P, D], F32, name=f"av_{qt}_{b}", tag="work")
                for kt in range(KT):
                    nc.tensor.matmul(
                        av_ps[:],
                        expT[:, (kt * BL + b) * NQ + qt * P:(kt * BL + b) * NQ + (qt + 1) * P],
                        vb[:, (b * KT + kt) * D:(b * KT + kt + 1) * D],
                        start=(kt == 0), stop=(kt == KT - 1))
                nc.vector.tensor_scalar_mul(
                    out_sb[:, (b * QT + qt) * D:(b * QT + qt + 1) * D],
                    av_ps[:],
                    recip_sb[:, b * QT + qt:b * QT + qt + 1])
                nc.sync.dma_start(out_d[b, qt * P:(qt + 1) * P, :],
                                  out_sb[:, (b * QT + qt) * D:(b * QT + qt + 1) * D])

    nc.compile()
    return nc


def _host_tables(b1: np.ndarray, w2: np.ndarray):
    """Tiny per-partition tables derived from the weight vectors."""
    phaseg = np.zeros((P, M_SINES * DC), np.float32)
    w2c = np.zeros((P, M_SINES * DC), np.float32)
    for mi in range(M_SINES):
        for dt in range(DC):
            col = mi * DC + dt
            phaseg[:, col] = NU[mi] * b1[dt * P:(dt + 1) * P]
            w2c[:, col] = COEF[mi] * w2[dt * P:(dt + 1) * P]
    return phaseg, w2c


_NC_CACHE = {}


def _get_nc():
    if "nc" not in _NC_CACHE:
        _NC_CACHE["nc"] = build_kernel()
    return _NC_CACHE["nc"]


def _make_in_maps(inputs):
    keys = np.ascontiguousarray(inputs["keys"], np.float32)
    queries = np.ascontiguousarray(inputs["queries"], np.float32)
    values = np.ascontiguousarray(inputs["values"], np.float32)
    Wk = np.ascontiguousarray(inputs["Wk"], np.float32)
    Wq = np.ascontiguousarray(inputs["Wq"], np.float32)
    b1 = np.asarray(inputs["b1"], np.float64)
    w2 = np.asarray(inputs["w2"], np.float64)
    phaseg, w2c = _host_tables(b1, w2)

    in_maps = []
    for c in range(NCORES):
        sl = slice(c * BL, (c + 1) * BL)
        in_maps.append({
            "queries": queries[sl], "keys": keys[sl], "values": values[sl],
            "Wq": Wq, "Wk": Wk, "phaseg": phaseg, "w2c": w2c,
        })
    return in_maps


def _run(inputs, trace=False, trace_kwargs=None):
    nc = _get_nc()
    in_maps = _make_in_maps(inputs)
    kwargs = {}
    if trace:
        kwargs = dict(trace=True, trace_cores=[0], trace_kwargs=trace_kwargs or {})
    res = run_bass_kernel_spmd(nc, in_maps, core_ids=list(range(NCORES)), **kwargs)
    out = np.concatenate([res.results[c]["out"] for c in range(NCORES)], axis=0)
    return out, res


def kernel(**inputs) -> np.ndarray:
    out, _ = _run(inputs, trace=False)
    return out


# revision 10
# speedup vs baseline: 1.2395x; 1.2395x over previous
"""Additive (Bahdanau) attention on 8 TRN2 NeuronCores — self-contained Bass kernel.

Math: score(q,k) = w2 . tanh(hq[q] + hk[k] + b1) + b2;  out = softmax_k(score) @ V.

Key restructuring: tanh(s) is approximated by a 6-term free-frequency sine
series  tanh(s) ~= sum_m c_m sin(w_m s)  (weighted-LSQ fit, rms 3.4e-4 under
the input law).  Then  sin(w(a+b)) = sin(wa)cos(wb) + cos(wa)sin(wb)  turns
the whole [B,Q,K,D] tanh+reduce into TensorE matmuls with contraction over
(2M x D):  logits^T[k,q] = sum_{m,d} G[(m,d),k] * F[(m,d),q].

HW Sin is only accurate on [-pi, pi]:
 - m=0 (w=0.31): |w h| < 1.4, no reduction; cos via sin(w h + pi/2).
 - m=1 (w=0.95): |w h| <~ 4, sin direct; cos = sin(pi/2 - |w h|) via an
   ACT Abs pass (arg in [-2.5, pi/2]).
 - m>=2: range-reduce on VectorE: t = nu*h (turns), r = round(t) via the
   +-1.5*2^23 magic add, fs = t - r in [-0.5,0.5] (TensorTensor subtract);
   then sin(2*pi*fs), and cos = sin(pi/2 - |2*pi*fs|) via ACT Abs.
b2 drops (softmax shift invariance); b1 folds into the hk PSUM->SBUF copy
(per-partition add); w2 and c_m fold into a per-partition scale of F;
1/denominator folds into the final per-q scaling of attn@V (denominator via
a ones-matmul, reciprocal on VectorE).  Transposes and the hq/hk matmuls run
in bf16 (error negligible vs the 2e-2 budget); logits/attn matmuls in bf16
with fp32 PSUM accumulation.

Sharding: data-parallel over batch, B=16 -> 2 per core, no collectives.
"""

import math
from contextlib import ExitStack

import numpy as np
import ml_dtypes

import concourse.bass as bass
import concourse.mybir as mybir
import concourse.tile as tile
from concourse import bacc
from concourse.bass_utils import run_bass_kernel_spmd
from concourse.masks import make_identity

F32 = mybir.dt.float32
BF16 = mybir.dt.bfloat16
AF = mybir.ActivationFunctionType
ALU = mybir.AluOpType

NCORES = 8
B, NQ, NK, D = 16, 256, 256, 256
BL = B // NCORES          # local batches per core = 2
P = 128
DC = D // P               # d-chunks = 2
EC = D // P               # e-chunks (contraction for hq/hk matmuls) = 2
QT = NQ // P              # q-tiles = 2
KT = NK // P              # k-tiles = 2
M_SINES = 6
TWO_PI = 2.0 * math.pi
MAGIC = 12582912.0        # 1.5 * 2**23: fp32 add/sub rounds to nearest integer
W = BL * NQ               # 512: free width per (dt) slice
WF = DC * W               # 1024: fused free width

# Free-frequency weighted-LSQ fit of tanh on [-8.5, 8.5] (Gaussian(1.05)+1e-3
# weight); max err 1.1e-2 on [-7,7], rms 3.4e-4 under the N(0,1) input law.
OMEGA = np.array([0.3126234509, 0.9450487939, 1.5943044008,
                  2.2642467516, 2.9676298346, 4.0923054586])
COEF = np.array([1.2258418724, 0.3056790312, 0.1089743779,
                 0.0378162937, 0.0157983516, 0.0046812861])
NU = OMEGA / TWO_PI       # "turns" multiplier
NO_RED = 1                # first NO_RED frequencies skip range reduction


def build_kernel() -> bacc.Bacc:
    nc = bacc.Bacc("TRN2", target_bir_lowering=False, debug=False)

    q_d = nc.dram_tensor("queries", [BL, NQ, D], F32, kind="ExternalInput").ap()
    k_d = nc.dram_tensor("keys", [BL, NK, D], F32, kind="ExternalInput").ap()
    v_d = nc.dram_tensor("values", [BL, NK, D], F32, kind="ExternalInput").ap()
    wq_d = nc.dram_tensor("Wq", [D, D], BF16, kind="ExternalInput").ap()
    wk_d = nc.dram_tensor("Wk", [D, D], BF16, kind="ExternalInput").ap()
    b1c_d = nc.dram_tensor("b1col", [P, DC], F32, kind="ExternalInput").ap()
    w2c_d = nc.dram_tensor("w2c", [P, M_SINES * DC], F32, kind="ExternalInput").ap()
    out_d = nc.dram_tensor("out", [BL, NQ, D], F32, kind="ExternalOutput").ap()

    with tile.TileContext(nc) as tc, ExitStack() as ctx:
        cpool = ctx.enter_context(tc.tile_pool(name="consts", bufs=1))
        dpool = ctx.enter_context(tc.tile_pool(name="data", bufs=1))

        ident = cpool.tile([P, P], BF16)
        make_identity(nc, ident[:])
        ones_bf = cpool.tile([P, 1], BF16)
        nc.gpsimd.memset(ones_bf[:], 1.0)
        halfpi = cpool.tile([P, 1], F32)
        nc.gpsimd.memset(halfpi[:], math.pi / 2.0)

        wq_sb = cpool.tile([P, EC * D], BF16)
        wk_sb = cpool.tile([P, EC * D], BF16)
        for ec in range(EC):
            nc.scalar.dma_start(wq_sb[:, ec * D:(ec + 1) * D], wq_d[ec * P:(ec + 1) * P, :])
            nc.scalar.dma_start(wk_sb[:, ec * D:(ec + 1) * D], wk_d[ec * P:(ec + 1) * P, :])
        b1col = cpool.tile([P, DC], F32)
        nc.scalar.dma_start(b1col[:], b1c_d[:])
        w2c = cpool.tile([P, M_SINES * DC], F32)
        nc.scalar.dma_start(w2c[:], w2c_d[:])

        # natural-layout loads: col = (b*2 + tile)*256 + inner
        qn = dpool.tile([P, BL * QT * D], F32)
        kn = dpool.tile([P, BL * KT * D], F32)
        vn = dpool.tile([P, BL * KT * D], F32)
        for b in range(BL):
            for t in range(QT):
                nc.sync.dma_start(qn[:, (b * QT + t) * D:(b * QT + t + 1) * D],
                                  q_d[b, t * P:(t + 1) * P, :])
            for t in range(KT):
                nc.sync.dma_start(kn[:, (b * KT + t) * D:(b * KT + t + 1) * D],
                                  k_d[b, t * P:(t + 1) * P, :])
                nc.scalar.dma_start(vn[:, (b * KT + t) * D:(b * KT + t + 1) * D],
                                    v_d[b, t * P:(t + 1) * P, :])

        # bf16 copies of q/k for the transpose + h matmuls
        qnb = dpool.tile([P, BL * QT * D], BF16)
        nc.vector.tensor_copy(qnb[:], qn[:])
        knb = dpool.tile([P, BL * KT * D], BF16)
        nc.vector.tensor_copy(knb[:], kn[:])

        # transposed inputs (bf16): col = (ec*BL + b)*256 + q
        qTt = dpool.tile([P, EC * BL * NQ], BF16)
        kTt = dpool.tile([P, EC * BL * NK], BF16)

        # h in fp32, dt-fused: col = dt*512 + b*256 + q
        hq_all = dpool.tile([P, WF], F32)
        hk_all = dpool.tile([P, WF], F32)

        with tc.tile_pool(name="hpsum", bufs=4, space="PSUM") as hpool:
            with tc.tile_pool(name="tpsum", bufs=4, space="PSUM") as tpool:
                for (src, dst, nt) in ((qnb, qTt, QT), (knb, kTt, KT)):
                    for b in range(BL):
                        for j in range(EC):
                            tp = tpool.tile([P, 2 * P], BF16, name="tp", tag="tp")
                            for i in range(nt):
                                nc.tensor.transpose(
                                    tp[:, i * P:(i + 1) * P],
                                    src[:, (b * nt + i) * D + j * P:(b * nt + i) * D + (j + 1) * P],
                                    ident[:])
                            nc.vector.tensor_copy(
                                dst[:, (j * BL + b) * NQ:(j * BL + b + 1) * NQ],
                                tp[:])

            for (w_sb, srcT, h_all, badd, n) in (
                    (wq_sb, qTt, hq_all, None, NQ), (wk_sb, kTt, hk_all, b1col, NK)):
                for dt in range(DC):
                    h_ps = hpool.tile([P, BL * n], F32, name="h_ps", tag="h_ps")
                    for b in range(BL):
                        for ec in range(EC):
                            nc.tensor.matmul(
                                h_ps[:, b * n:(b + 1) * n],
                                w_sb[:, ec * D + dt * P:ec * D + (dt + 1) * P],
                                srcT[:, (ec * BL + b) * n:(ec * BL + b + 1) * n],
                                start=(ec == 0), stop=(ec == EC - 1))
                    if badd is None:
                        nc.vector.tensor_copy(h_all[:, dt * W:(dt + 1) * W], h_ps[:])
                    else:
                        nc.vector.tensor_scalar(h_all[:, dt * W:(dt + 1) * W], h_ps[:],
                                                badd[:, dt:dt + 1], None, op0=ALU.add)

        wpool = ctx.enter_context(tc.tile_pool(name="wpsum", bufs=4, space="PSUM"))
        dnpool = ctx.enter_context(tc.tile_pool(name="dnpsum", bufs=2, space="PSUM"))
        tfpool = ctx.enter_context(tc.tile_pool(name="turns", bufs=2))
        frpool = ctx.enter_context(tc.tile_pool(name="fracs", bufs=2))
        rpool = ctx.enter_context(tc.tile_pool(name="raws", bufs=3))
        s1pool = ctx.enter_context(tc.tile_pool(name="scaledF", bufs=3))

        # logits^T accumulation: tile per (k-tile, batch) — a PSUM accumulation
        # group claims a whole 2KB bank, so concurrent groups get separate tiles
        logits_ps = [[wpool.tile([P, NQ], F32, name=f"lg_{kt}_{b}", tag="work")
                      for b in range(BL)] for kt in range(KT)]

        first = True
        for mi in range(M_SINES):
            omega = float(OMEGA[mi])
            nu = float(NU[mi])
            last = (mi == M_SINES - 1)
            sides = []
            for (tag, h_all) in (("F", hq_all), ("G", hk_all)):
                sn = rpool.tile([P, WF], BF16, name=f"sn{tag}", tag=f"sn{tag}")
                cs = rpool.tile([P, WF], BF16, name=f"cs{tag}", tag=f"cs{tag}")
                if mi == 0:
                    # |w h| < pi/2: sin direct, cos via +pi/2 shift
                    nc.scalar.activation(sn[:], h_all[:], AF.Sin, bias=0.0, scale=omega)
                    nc.scalar.activation(cs[:], h_all[:], AF.Sin, bias=halfpi[:], scale=omega)
                elif mi < NO_RED:
                    # |w h| <~ 4: sin direct; cos = sin(pi/2 - |w h|)
                    nc.scalar.activation(sn[:], h_all[:], AF.Sin, bias=0.0, scale=omega)
                    ab = frpool.tile([P, WF], F32, name=f"ab{tag}", tag=f"ab{tag}")
                    nc.scalar.activation(ab[:], h_all[:], AF.Abs, bias=0.0, scale=omega)
                    nc.scalar.activation(cs[:], ab[:], AF.Sin, bias=halfpi[:], scale=-1.0)
                else:
                    # full range reduction to fs in [-0.5, 0.5] turns
                    t = tfpool.tile([P, WF], F32, name=f"t{tag}", tag=f"t{tag}")
                    nc.vector.tensor_scalar(t[:], h_all[:], nu, None, op0=ALU.mult)
                    r = tfpool.tile([P, WF], F32, name=f"r{tag}", tag=f"r{tag}")
                    nc.vector.tensor_scalar(r[:], t[:], MAGIC, MAGIC,
                                            op0=ALU.add, op1=ALU.subtract)
                    fs = frpool.tile([P, WF], F32, name=f"fs{tag}", tag=f"fs{tag}")
                    nc.vector.tensor_tensor(fs[:], t[:], r[:], op=ALU.subtract)
                    nc.scalar.activation(sn[:], fs[:], AF.Sin, bias=0.0, scale=TWO_PI)
                    ab = frpool.tile([P, WF], F32, name=f"ab{tag}", tag=f"ab{tag}")
                    nc.scalar.activation(ab[:], fs[:], AF.Abs, bias=0.0, scale=TWO_PI)
                    nc.scalar.activation(cs[:], ab[:], AF.Sin, bias=halfpi[:], scale=-1.0)
                sides.append((sn, cs))
            (rFs, rFc), (rGs, rGc) = sides
            sF = s1pool.tile([P, 2 * WF], BF16, name="sF", tag="sF")
            for dt in range(DC):
                col = mi * DC + dt
                nc.vector.tensor_scalar_mul(sF[:, dt * W:(dt + 1) * W],
                                            rFs[:, dt * W:(dt + 1) * W],
                                            w2c[:, col:col + 1])
                nc.vector.tensor_scalar_mul(sF[:, WF + dt * W:WF + (dt + 1) * W],
                                            rFc[:, dt * W:(dt + 1) * W],
                                            w2c[:, col:col + 1])
            # logits += Gcos^T (w2c*Fsin) + Gsin^T (w2c*Fcos)
            for (pi_, gt) in ((0, rGc), (1, rGs)):
                for dt in range(DC):
                    for b in range(BL):
                        for kt in range(KT):
                            nc.tensor.matmul(
                                logits_ps[kt][b][:],
                                gt[:, dt * W + b * NQ + kt * P:dt * W + b * NQ + (kt + 1) * P],
                                sF[:, pi_ * WF + dt * W + b * NQ:pi_ * WF + dt * W + (b + 1) * NQ],
                                start=first, stop=(last and pi_ == 1 and dt == DC - 1))
                    first = False

        # exp(logits^T) -> bf16 SBUF, col = (kt*BL + b)*256 + q
        expT = dpool.tile([P, KT * BL * NQ], BF16)
        for kt in range(KT):
            for b in range(BL):
                nc.scalar.activation(
                    expT[:, (kt * BL + b) * NQ:(kt * BL + b + 1) * NQ],
                    logits_ps[kt][b][:], AF.Exp)

        # denominators as columns via ones-matmul (one bank per accumulation group)
        recip_sb = cpool.tile([P, BL * QT], F32)
        for b in range(BL):
            for qt in range(QT):
                dn = dnpool.tile([P, 1], F32, name=f"dn_{b}_{qt}", tag="dn")
                for kt in range(KT):
                    nc.tensor.matmul(
                        dn[:],
                        expT[:, (kt * BL + b) * NQ + qt * P:(kt * BL + b) * NQ + (qt + 1) * P],
                        ones_bf[:],
                        start=(kt == 0), stop=(kt == KT - 1))
                nc.vector.reciprocal(recip_sb[:, b * QT + qt:b * QT + qt + 1], dn[:])

        vb = dpool.tile([P, BL * KT * D], BF16)
        nc.vector.tensor_copy(vb[:], vn[:])

        # attn @ V (unnormalized), then fold in 1/denom per q-partition
        out_sb = dpool.tile([P, BL * QT * D], F32)
        for qt in range(QT):
            for b in range(BL):
                av_ps = wpool.tile([P, D], F32, name=f"av_{qt}_{b}", tag="work")
                for kt in range(KT):
                    nc.tensor.matmul(
                        av_ps[:],
                        expT[:, (kt * BL + b) * NQ + qt * P:(kt * BL + b) * NQ + (qt + 1) * P],
                        vb[:, (b * KT + kt) * D:(b * KT + kt + 1) * D],
                        start=(kt == 0), stop=(kt == KT - 1))
                nc.vector.tensor_scalar_mul(
                    out_sb[:, (b * QT + qt) * D:(b * QT + qt + 1) * D],
                    av_ps[:],
                    recip_sb[:, b * QT + qt:b * QT + qt + 1])
                nc.sync.dma_start(out_d[b, qt * P:(qt + 1) * P, :],
                                  out_sb[:, (b * QT + qt) * D:(b * QT + qt + 1) * D])

    nc.compile()
    return nc


def _host_tables(b1: np.ndarray, w2: np.ndarray):
    """Tiny per-partition tables derived from the weight vectors."""
    b1col = np.zeros((P, DC), np.float32)
    w2c = np.zeros((P, M_SINES * DC), np.float32)
    for dt in range(DC):
        b1col[:, dt] = b1[dt * P:(dt + 1) * P]
        for mi in range(M_SINES):
            w2c[:, mi * DC + dt] = COEF[mi] * w2[dt * P:(dt + 1) * P]
    return b1col, w2c


_NC_CACHE = {}


def _get_nc():
    if "nc" not in _NC_CACHE:
        _NC_CACHE["nc"] = build_kernel()
    return _NC_CACHE["nc"]


def _make_in_maps(inputs):
    keys = np.ascontiguousarray(inputs["keys"], np.float32)
    queries = np.ascontiguousarray(inputs["queries"], np.float32)
    values = np.ascontiguousarray(inputs["values"], np.float32)
    Wk = np.ascontiguousarray(np.asarray(inputs["Wk"], np.float32).astype(ml_dtypes.bfloat16))
    Wq = np.ascontiguousarray(np.asarray(inputs["Wq"], np.float32).astype(ml_dtypes.bfloat16))
    b1 = np.asarray(inputs["b1"], np.float64)
    w2 = np.asarray(inputs["w2"], np.float64)
    b1col, w2c = _host_tables(b1, w2)

    in_maps = []
    for c in range(NCORES):
        sl = slice(c * BL, (c + 1) * BL)
        in_maps.append({
            "queries": queries[sl], "keys": keys[sl], "values": values[sl],
            "Wq": Wq, "Wk": Wk, "b1col": b1col, "w2c": w2c,
        })
    return in_maps


def _run(inputs, trace=False, trace_kwargs=None):
    nc = _get_nc()
    in_maps = _make_in_maps(inputs)
    kwargs = {}
    if trace:
        kwargs = dict(trace=True, trace_cores=[0], trace_kwargs=trace_kwargs or {})
    res = run_bass_kernel_spmd(nc, in_maps, core_ids=list(range(NCORES)), **kwargs)
    out = np.concatenate([res.results[c]["out"] for c in range(NCORES)], axis=0)
    return out, res


def kernel(**inputs) -> np.ndarray:
    out, _ = _run(inputs, trace=False)
    return out


# revision 11
# speedup vs baseline: 1.3877x; 1.1195x over previous
"""Additive (Bahdanau) attention on 8 TRN2 NeuronCores — self-contained Bass kernel.

Math: score(q,k) = w2 . tanh(hq[q] + hk[k] + b1) + b2;  out = softmax_k(score) @ V.

Key restructuring: tanh(s) is approximated by a 6-term free-frequency sine
series  tanh(s) ~= sum_m c_m sin(w_m s)  (weighted-LSQ fit, rms 3.4e-4 under
the input law).  Then  sin(w(a+b)) = sin(wa)cos(wb) + cos(wa)sin(wb)  turns
the whole [B,Q,K,D] tanh+reduce into TensorE matmuls with contraction over
(2M x D):  logits^T[k,q] = sum_{m,d} G[(m,d),k] * F[(m,d),q].

HW Sin is only accurate on [-pi, pi]:
 - m=0 (w=0.31): |w h| < 1.4, no reduction; cos via sin(w h + pi/2).
 - m=1 (w=0.95): |w h| <~ 4, sin direct; cos = sin(pi/2 - |w h|) via an
   ACT Abs pass (arg in [-2.5, pi/2]).
 - m>=2: range-reduce on VectorE: t = nu*h (turns), r = round(t) via the
   +-1.5*2^23 magic add, fs = t - r in [-0.5,0.5] (TensorTensor subtract);
   then sin(2*pi*fs), and cos = sin(pi/2 - |2*pi*fs|) via ACT Abs.
b2 drops (softmax shift invariance); b1 folds into the hk PSUM->SBUF copy
(per-partition add); w2 and c_m fold into a per-partition scale of F;
1/denominator folds into the final per-q scaling of attn@V (denominator via
a ones-matmul, reciprocal on VectorE).  Transposes and the hq/hk matmuls run
in bf16 (error negligible vs the 2e-2 budget); logits/attn matmuls in bf16
with fp32 PSUM accumulation.

Sharding: data-parallel over batch, B=16 -> 2 per core, no collectives.
"""

import math
from contextlib import ExitStack

import numpy as np
import ml_dtypes

import concourse.bass as bass
import concourse.mybir as mybir
import concourse.tile as tile
from concourse import bacc
from concourse.bass_utils import run_bass_kernel_spmd
from concourse.masks import make_identity

F32 = mybir.dt.float32
BF16 = mybir.dt.bfloat16
AF = mybir.ActivationFunctionType
ALU = mybir.AluOpType

NCORES = 8
B, NQ, NK, D = 16, 256, 256, 256
BL = B // NCORES          # local batches per core = 2
P = 128
DC = D // P               # d-chunks = 2
EC = D // P               # e-chunks (contraction for hq/hk matmuls) = 2
QT = NQ // P              # q-tiles = 2
KT = NK // P              # k-tiles = 2
M_SINES = 5
TWO_PI = 2.0 * math.pi
MAGIC = 12582912.0        # 1.5 * 2**23: fp32 add/sub rounds to nearest integer
W = BL * NQ               # 512: free width per (dt) slice
WF = DC * W               # 1024: fused free width

# Free-frequency weighted-LSQ fit of tanh on [-8.5, 8.5] (Gaussian(1.05)+1e-3
# weight); e2e error vs the fp64 reference is ~3e-3 (bf16-noise dominated).
OMEGA = np.array([0.3163285035, 0.9562143912, 1.6122028962,
                  2.3151891254, 3.4784821142])
COEF = np.array([1.224359211, 0.3049242901, 0.1033682241,
                 0.0452539231, 0.0126629248])
NU = OMEGA / TWO_PI       # "turns" multiplier
NO_RED = 1                # first NO_RED frequencies skip range reduction


def build_kernel() -> bacc.Bacc:
    nc = bacc.Bacc("TRN2", target_bir_lowering=False, debug=False)

    q_d = nc.dram_tensor("queries", [BL, NQ, D], F32, kind="ExternalInput").ap()
    k_d = nc.dram_tensor("keys", [BL, NK, D], F32, kind="ExternalInput").ap()
    v_d = nc.dram_tensor("values", [BL, NK, D], F32, kind="ExternalInput").ap()
    wq_d = nc.dram_tensor("Wq", [D, D], BF16, kind="ExternalInput").ap()
    wk_d = nc.dram_tensor("Wk", [D, D], BF16, kind="ExternalInput").ap()
    b1c_d = nc.dram_tensor("b1col", [P, DC], F32, kind="ExternalInput").ap()
    w2c_d = nc.dram_tensor("w2c", [P, M_SINES * DC], F32, kind="ExternalInput").ap()
    out_d = nc.dram_tensor("out", [BL, NQ, D], F32, kind="ExternalOutput").ap()

    with tile.TileContext(nc) as tc, ExitStack() as ctx:
        cpool = ctx.enter_context(tc.tile_pool(name="consts", bufs=1))
        dpool = ctx.enter_context(tc.tile_pool(name="data", bufs=1))

        ident = cpool.tile([P, P], BF16)
        make_identity(nc, ident[:])
        ones_bf = cpool.tile([P, 1], BF16)
        nc.gpsimd.memset(ones_bf[:], 1.0)
        halfpi = cpool.tile([P, 1], F32)
        nc.gpsimd.memset(halfpi[:], math.pi / 2.0)

        wq_sb = cpool.tile([P, EC * D], BF16)
        wk_sb = cpool.tile([P, EC * D], BF16)
        for ec in range(EC):
            nc.scalar.dma_start(wq_sb[:, ec * D:(ec + 1) * D], wq_d[ec * P:(ec + 1) * P, :])
            nc.scalar.dma_start(wk_sb[:, ec * D:(ec + 1) * D], wk_d[ec * P:(ec + 1) * P, :])
        b1col = cpool.tile([P, DC], F32)
        nc.scalar.dma_start(b1col[:], b1c_d[:])
        w2c = cpool.tile([P, M_SINES * DC], F32)
        nc.scalar.dma_start(w2c[:], w2c_d[:])

        # natural-layout loads: col = (b*2 + tile)*256 + inner
        qn = dpool.tile([P, BL * QT * D], F32)
        kn = dpool.tile([P, BL * KT * D], F32)
        vn = dpool.tile([P, BL * KT * D], F32)
        for b in range(BL):
            for t in range(QT):
                nc.sync.dma_start(qn[:, (b * QT + t) * D:(b * QT + t + 1) * D],
                                  q_d[b, t * P:(t + 1) * P, :])
        for b in range(BL):
            for t in range(KT):
                nc.sync.dma_start(kn[:, (b * KT + t) * D:(b * KT + t + 1) * D],
                                  k_d[b, t * P:(t + 1) * P, :])
                nc.scalar.dma_start(vn[:, (b * KT + t) * D:(b * KT + t + 1) * D],
                                    v_d[b, t * P:(t + 1) * P, :])

        qnb = dpool.tile([P, BL * QT * D], BF16)
        knb = dpool.tile([P, BL * KT * D], BF16)

        # transposed inputs (bf16): col = (ec*BL + b)*256 + q
        qTt = dpool.tile([P, EC * BL * NQ], BF16)
        kTt = dpool.tile([P, EC * BL * NK], BF16)

        # h in fp32, side+dt-fused: F (queries) at col dt*512 + b*256 + q,
        # G (keys, +b1) at col 1024 + dt*512 + b*256 + k
        h_both = dpool.tile([P, 2 * WF], F32)

        with tc.tile_pool(name="hpsum", bufs=4, space="PSUM") as hpool:
            with tc.tile_pool(name="tpsum", bufs=4, space="PSUM") as tpool:
                # full q pipeline first so ScalarE can start m=0 early;
                # k pipeline follows (PE executes in program order)
                for (nat, natb, dst, w_sb, badd, off, nt, n) in (
                        (qn, qnb, qTt, wq_sb, None, 0, QT, NQ),
                        (kn, knb, kTt, wk_sb, b1col, WF, KT, NK)):
                    for b in range(BL):
                        for t in range(nt):
                            nc.vector.tensor_copy(
                                natb[:, (b * nt + t) * D:(b * nt + t + 1) * D],
                                nat[:, (b * nt + t) * D:(b * nt + t + 1) * D])
                    for b in range(BL):
                        for j in range(EC):
                            tp = tpool.tile([P, 2 * P], BF16, name="tp", tag="tp")
                            for i in range(nt):
                                nc.tensor.transpose(
                                    tp[:, i * P:(i + 1) * P],
                                    natb[:, (b * nt + i) * D + j * P:(b * nt + i) * D + (j + 1) * P],
                                    ident[:])
                            nc.vector.tensor_copy(
                                dst[:, (j * BL + b) * NQ:(j * BL + b + 1) * NQ],
                                tp[:])
                    for dt in range(DC):
                        h_ps = hpool.tile([P, BL * n], F32, name="h_ps", tag="h_ps")
                        for b in range(BL):
                            for ec in range(EC):
                                nc.tensor.matmul(
                                    h_ps[:, b * n:(b + 1) * n],
                                    w_sb[:, ec * D + dt * P:ec * D + (dt + 1) * P],
                                    dst[:, (ec * BL + b) * n:(ec * BL + b + 1) * n],
                                    start=(ec == 0), stop=(ec == EC - 1))
                        if badd is None:
                            nc.vector.tensor_copy(h_both[:, off + dt * W:off + (dt + 1) * W], h_ps[:])
                        else:
                            nc.vector.tensor_scalar(h_both[:, off + dt * W:off + (dt + 1) * W],
                                                    h_ps[:], badd[:, dt:dt + 1], None, op0=ALU.add)

        wpool = ctx.enter_context(tc.tile_pool(name="wpsum", bufs=4, space="PSUM"))
        dnpool = ctx.enter_context(tc.tile_pool(name="dnpsum", bufs=2, space="PSUM"))
        tfpool = ctx.enter_context(tc.tile_pool(name="turns", bufs=2))
        frpool = ctx.enter_context(tc.tile_pool(name="fracs", bufs=2))
        rpool = ctx.enter_context(tc.tile_pool(name="raws", bufs=3))
        s1pool = ctx.enter_context(tc.tile_pool(name="scaledF", bufs=3))

        # logits^T accumulation: tile per (k-tile, batch) — a PSUM accumulation
        # group claims a whole 2KB bank, so concurrent groups get separate tiles
        logits_ps = [[wpool.tile([P, NQ], F32, name=f"lg_{kt}_{b}", tag="work")
                      for b in range(BL)] for kt in range(KT)]

        first = True
        for mi in range(M_SINES):
            omega = float(OMEGA[mi])
            nu = float(NU[mi])
            last = (mi == M_SINES - 1)
            # sin/cos of both sides in single [128, 2048] ops
            sn = rpool.tile([P, 2 * WF], BF16, name="sn", tag="sn")
            cs = rpool.tile([P, 2 * WF], BF16, name="cs", tag="cs")
            if mi < NO_RED:
                # |w h| < 1.4: sin direct, cos via +pi/2 shift (stays in domain)
                nc.scalar.activation(sn[:], h_both[:], AF.Sin, bias=0.0, scale=omega)
                nc.scalar.activation(cs[:], h_both[:], AF.Sin, bias=halfpi[:], scale=omega)
            else:
                # full range reduction to fs in [-0.5, 0.5] turns
                t = tfpool.tile([P, 2 * WF], F32, name="t", tag="t")
                nc.vector.tensor_scalar(t[:], h_both[:], nu, None, op0=ALU.mult)
                r = tfpool.tile([P, 2 * WF], F32, name="r", tag="r")
                nc.vector.tensor_scalar(r[:], t[:], MAGIC, MAGIC,
                                        op0=ALU.add, op1=ALU.subtract)
                fs = frpool.tile([P, 2 * WF], F32, name="fs", tag="fs")
                nc.vector.tensor_tensor(fs[:], t[:], r[:], op=ALU.subtract)
                nc.scalar.activation(sn[:], fs[:], AF.Sin, bias=0.0, scale=TWO_PI)
                ab = frpool.tile([P, 2 * WF], F32, name="ab", tag="ab")
                nc.scalar.activation(ab[:], fs[:], AF.Abs, bias=0.0, scale=TWO_PI)
                nc.scalar.activation(cs[:], ab[:], AF.Sin, bias=halfpi[:], scale=-1.0)
            sF = s1pool.tile([P, 2 * WF], BF16, name="sF", tag="sF")
            for dt in range(DC):
                col = mi * DC + dt
                nc.vector.tensor_scalar_mul(sF[:, dt * W:(dt + 1) * W],
                                            sn[:, dt * W:(dt + 1) * W],
                                            w2c[:, col:col + 1])
                nc.vector.tensor_scalar_mul(sF[:, WF + dt * W:WF + (dt + 1) * W],
                                            cs[:, dt * W:(dt + 1) * W],
                                            w2c[:, col:col + 1])
            # logits += Gcos^T (w2c*Fsin) + Gsin^T (w2c*Fcos)
            for (pi_, gt) in ((0, cs), (1, sn)):
                for dt in range(DC):
                    for b in range(BL):
                        for kt in range(KT):
                            nc.tensor.matmul(
                                logits_ps[kt][b][:],
                                gt[:, WF + dt * W + b * NQ + kt * P:WF + dt * W + b * NQ + (kt + 1) * P],
                                sF[:, pi_ * WF + dt * W + b * NQ:pi_ * WF + dt * W + (b + 1) * NQ],
                                start=first, stop=(last and pi_ == 1 and dt == DC - 1))
                    first = False

        # exp(logits^T) -> bf16 SBUF, col = (kt*BL + b)*256 + q
        expT = dpool.tile([P, KT * BL * NQ], BF16)
        for kt in range(KT):
            for b in range(BL):
                nc.scalar.activation(
                    expT[:, (kt * BL + b) * NQ:(kt * BL + b + 1) * NQ],
                    logits_ps[kt][b][:], AF.Exp)

        # denominators as columns via ones-matmul (one bank per accumulation group)
        recip_sb = cpool.tile([P, BL * QT], F32)
        for b in range(BL):
            for qt in range(QT):
                dn = dnpool.tile([P, 1], F32, name=f"dn_{b}_{qt}", tag="dn")
                for kt in range(KT):
                    nc.tensor.matmul(
                        dn[:],
                        expT[:, (kt * BL + b) * NQ + qt * P:(kt * BL + b) * NQ + (qt + 1) * P],
                        ones_bf[:],
                        start=(kt == 0), stop=(kt == KT - 1))
                nc.vector.reciprocal(recip_sb[:, b * QT + qt:b * QT + qt + 1], dn[:])

        vb = dpool.tile([P, BL * KT * D], BF16)
        nc.vector.tensor_copy(vb[:], vn[:])

        # attn @ V (unnormalized), then fold in 1/denom per q-partition
        out_sb = dpool.tile([P, BL * QT * D], F32)
        for qt in range(QT):
            for b in range(BL):
                av_ps = wpool.tile([P, D], F32, name=f"av_{qt}_{b}", tag="work")
                for kt in range(KT):
                    nc.tensor.matmul(
                        av_ps[:],
                        expT[:, (kt * BL + b) * NQ + qt * P:(kt * BL + b) * NQ + (qt + 1) * P],
                        vb[:, (b * KT + kt) * D:(b * KT + kt + 1) * D],
                        start=(kt == 0), stop=(kt == KT - 1))
                nc.vector.tensor_scalar_mul(
                    out_sb[:, (b * QT + qt) * D:(b * QT + qt + 1) * D],
                    av_ps[:],
                    recip_sb[:, b * QT + qt:b * QT + qt + 1])
                nc.sync.dma_start(out_d[b, qt * P:(qt + 1) * P, :],
                                  out_sb[:, (b * QT + qt) * D:(b * QT + qt + 1) * D])

    nc.compile()
    return nc


def _host_tables(b1: np.ndarray, w2: np.ndarray):
    """Tiny per-partition tables derived from the weight vectors."""
    b1col = np.zeros((P, DC), np.float32)
    w2c = np.zeros((P, M_SINES * DC), np.float32)
    for dt in range(DC):
        b1col[:, dt] = b1[dt * P:(dt + 1) * P]
        for mi in range(M_SINES):
            w2c[:, mi * DC + dt] = COEF[mi] * w2[dt * P:(dt + 1) * P]
    return b1col, w2c


_NC_CACHE = {}


def _get_nc():
    if "nc" not in _NC_CACHE:
        _NC_CACHE["nc"] = build_kernel()
    return _NC_CACHE["nc"]


def _make_in_maps(inputs):
    keys = np.ascontiguousarray(inputs["keys"], np.float32)
    queries = np.ascontiguousarray(inputs["queries"], np.float32)
    values = np.ascontiguousarray(inputs["values"], np.float32)
    Wk = np.ascontiguousarray(np.asarray(inputs["Wk"], np.float32).astype(ml_dtypes.bfloat16))
    Wq = np.ascontiguousarray(np.asarray(inputs["Wq"], np.float32).astype(ml_dtypes.bfloat16))
    b1 = np.asarray(inputs["b1"], np.float64)
    w2 = np.asarray(inputs["w2"], np.float64)
    b1col, w2c = _host_tables(b1, w2)

    in_maps = []
    for c in range(NCORES):
        sl = slice(c * BL, (c + 1) * BL)
        in_maps.append({
            "queries": queries[sl], "keys": keys[sl], "values": values[sl],
            "Wq": Wq, "Wk": Wk, "b1col": b1col, "w2c": w2c,
        })
    return in_maps


def _run(inputs, trace=False, trace_kwargs=None):
    nc = _get_nc()
    in_maps = _make_in_maps(inputs)
    kwargs = {}
    if trace:
        kwargs = dict(trace=True, trace_cores=[0], trace_kwargs=trace_kwargs or {})
    res = run_bass_kernel_spmd(nc, in_maps, core_ids=list(range(NCORES)), **kwargs)
    out = np.concatenate([res.results[c]["out"] for c in range(NCORES)], axis=0)
    return out, res


def kernel(**inputs) -> np.ndarray:
    out, _ = _run(inputs, trace=False)
    return out
